# revision 5
# baseline (speedup 1.0000x reference)
"""NodeUnpool kernel for 8 Trainium2 NeuronCores (Bass/Tile, SPMD).

Computation (see nn.Module reference):
    old = h_full[old_idxs]                      # [M, 256] gather
    merged = old @ W1.T + b1 + h_sub @ W2.T + b2
    out = h_full with rows old_idxs replaced by merged

Strategy:
  * old_idxs is arange(M) in this problem (fill="arange"), so the gather and
    scatter are contiguous row slices. A general host-side gather/scatter
    fallback handles any other index pattern.
  * The device work is exactly the merged-row GEMM: X=[old | h_sub] [M,512]
    @ Wc.T + (b1+b2), sharded row-wise across 8 cores (M/8 = 31250 rows each).
  * Activations are fed feature-major (host pre-transpose) so the 512-deep
    contraction lies on SBUF partitions: outT[j,r] = sum_k Wc.T[k,j] * X.T[k,r].
    Weights are the stationary operand; PSUM accumulates 4 k-tiles; the bias is
    added during PSUM->SBUF eviction (per-partition scalar on the DVE).
  * bf16 activations/weights/outputs: PE runs at the same 1 cycle/row as
    fp32r (ap_size >= 256), but HBM traffic halves vs fp32 (49 MB vs 98 MB
    per core), keeping the kernel PE-bound (~254k PE cycles @ 2.4 GHz =
    ~106 us) even on slower-DMA parts. Max rel-err vs fp32 ~4e-3 (gate 2e-2).
  * No column padding: each core processes exactly 31250 rows (last chunk is
    530 wide, last PSUM block 18 wide).
  * Pass-through rows (h_full[M:]) never touch the device; they are copied on
    the host during output assembly.
"""

import sys
from concurrent.futures import ThreadPoolExecutor

import numpy as np
import ml_dtypes

BF16 = ml_dtypes.bfloat16

N, M, DIM = 1_000_000, 256 * 1024 - 12144, 256  # M = 250_000
M = 250_000
N_CORES = 8
ROWS_PC = M // N_CORES  # 31250 merged rows per core
CHUNK = 2048            # columns (rows of X) processed per inner step
KT = (2 * DIM) // 128   # 4 contraction tiles
JT = DIM // 128         # 2 output-feature blocks

_NC_CACHE = {}
_POOL = ThreadPoolExecutor(max_workers=N_CORES)


def _ensure_concourse():
    try:
        import concourse.bass  # noqa: F401
    except ImportError:  # pragma: no cover
        sys.path.insert(0, "/opt/trn_rl_repo")
        import concourse.bass  # noqa: F401


def _build_nc(in_dt="bfloat16", reps=1):
    """Build + bacc-compile the per-core Bass program (identical on all cores).

    reps > 1 repeats the whole compute loop inside one NEFF (used only by
    dev_hwtime's slope measurement; the shipped kernel uses reps=1).
    """
    _ensure_concourse()
    import concourse.bacc as bacc
    import concourse.tile as tile
    from concourse import mybir

    dt_in = getattr(mybir.dt, in_dt)
    f32 = mybir.dt.float32

    nc = bacc.Bacc("TRN2", target_bir_lowering=False, debug=False)
    xT = nc.dram_tensor("xT", [2 * DIM, ROWS_PC], dt_in, kind="ExternalInput")
    wT = nc.dram_tensor("wT", [2 * DIM, DIM], dt_in, kind="ExternalInput")
    bias = nc.dram_tensor("bias", [128, JT], f32, kind="ExternalInput")
    # one output tensor per rep so no rep's stores are dead (reps>1 is only
    # used for timing; the shipped kernel has reps=1 and a single "outT")
    outTs = [
        nc.dram_tensor(
            "outT" if r == 0 else f"outT{r}", [DIM, ROWS_PC], dt_in,
            kind="ExternalOutput",
        )
        for r in range(reps)
    ]

    with tile.TileContext(nc) as tc:
        with (
            tc.tile_pool(name="wpool", bufs=1) as wpool,
            tc.tile_pool(name="io", bufs=2) as io,
            tc.tile_pool(name="pp", bufs=4, space="PSUM") as pp,
        ):
            w_sb = wpool.tile([128, KT * DIM], dt_in)
            for kt in range(KT):
                nc.sync.dma_start(
                    out=w_sb[:, kt * DIM : (kt + 1) * DIM],
                    in_=wT[kt * 128 : (kt + 1) * 128, :],
                )
            b_sb = wpool.tile([128, JT], f32)
            nc.sync.dma_start(out=b_sb[:], in_=bias[:])

            for _rep in range(reps):
                outT = outTs[_rep]
                col = 0
                while col < ROWS_PC:
                    ch = min(CHUNK, ROWS_PC - col)
                    xts = []
                    for kt in range(KT):
                        xtile = io.tile([128, CHUNK], dt_in, tag=f"x{kt}", name=f"x{kt}")
                        nc.sync.dma_start(
                            out=xtile[:, :ch],
                            in_=xT[kt * 128 : (kt + 1) * 128, col : col + ch],
                        )
                        xts.append(xtile)
                    for j2 in range(JT):
                        ot = io.tile([128, CHUNK], dt_in, tag=f"o{j2}", name=f"o{j2}")
                        for n in range(0, ch, 512):
                            nsz = min(512, ch - n)
                            ps = pp.tile([128, 512], f32, tag="ps", name="ps")
                            for kt in range(KT):
                                nc.tensor.matmul(
                                    ps[:, :nsz],
                                    w_sb[:, kt * DIM + j2 * 128 : kt * DIM + j2 * 128 + 128],
                                    xts[kt][:, n : n + nsz],
                                    start=(kt == 0),
                                    stop=(kt == KT - 1),
                                )
                            nc.vector.tensor_scalar_add(
                                ot[:, n : n + nsz], ps[:, :nsz], b_sb[:, j2 : j2 + 1]
                            )
                        nc.sync.dma_start(
                            out=outT[j2 * 128 : (j2 + 1) * 128, col : col + ch],
                            in_=ot[:, :ch],
                        )
                    col += ch
    nc.compile()
    return nc


def _get_nc(in_dt="bfloat16"):
    if in_dt not in _NC_CACHE:
        _NC_CACHE[in_dt] = _build_nc(in_dt)
    return _NC_CACHE[in_dt]


_TBLK = 256  # row-block size for cache-friendly host transposes


def _make_core_input(xm_bf, h_sub_bf, c):
    """Per-core feature-major activation block [512, ROWS_PC] bf16."""
    lo, hi = c * ROWS_PC, (c + 1) * ROWS_PC
    xT_c = np.empty((2 * DIM, ROWS_PC), BF16)
    for i in range(lo, hi, _TBLK):
        j = min(i + _TBLK, hi)
        xT_c[:DIM, i - lo : j - lo] = xm_bf[i:j].T
        xT_c[DIM:, i - lo : j - lo] = h_sub_bf[i:j].T
    return xT_c


def _run_device(in_maps):
    _ensure_concourse()
    from concourse.bass_utils import run_bass_kernel_spmd

    nc = _get_nc()
    return run_bass_kernel_spmd(nc, in_maps, list(range(N_CORES))).results


def _copy_rows(dst, src, lo, hi):
    np.copyto(dst[lo:hi], src[lo:hi])


def kernel(h_full, h_sub, W1, b1, W2, b2, old_idxs):
    h_full = np.asarray(h_full, dtype=np.float32)
    h_sub = np.asarray(h_sub, dtype=np.float32)
    W1 = np.asarray(W1, dtype=np.float32)
    W2 = np.asarray(W2, dtype=np.float32)
    b1 = np.asarray(b1, dtype=np.float32)
    b2 = np.asarray(b2, dtype=np.float32)
    idx = np.asarray(old_idxs)

    fast = idx.shape == (M,) and bool(
        np.array_equal(idx, np.arange(M, dtype=idx.dtype))
    )
    xm = h_full[:M] if fast else np.ascontiguousarray(h_full[idx])

    wT = np.concatenate([W1.T, W2.T], axis=0).astype(BF16)
    bias = np.ascontiguousarray((b1 + b2).astype(np.float32).reshape(JT, 128).T)

    # contiguous fp32 -> bf16 casts (fast), then 2-byte blocked transposes
    xm_bf = None
    h_sub_bf = None

    def _cast_rows(src, c):
        lo, hi = c * ROWS_PC, (c + 1) * ROWS_PC
        return src[lo:hi].astype(BF16)

    xm_parts = list(_POOL.map(lambda c: _cast_rows(xm, c), range(N_CORES)))
    sub_parts = list(_POOL.map(lambda c: _cast_rows(h_sub, c), range(N_CORES)))
    xm_bf = np.concatenate(xm_parts, axis=0)
    h_sub_bf = np.concatenate(sub_parts, axis=0)

    xTs = list(
        _POOL.map(lambda c: _make_core_input(xm_bf, h_sub_bf, c), range(N_CORES))
    )
    in_maps = [{"xT": xTs[c], "wT": wT, "bias": bias} for c in range(N_CORES)]

    results = _run_device(in_maps)

    out = np.empty((N, DIM), np.float32)

    def _untranspose_into(dst, src_t):
        rows = dst.shape[0]
        for i in range(0, rows, _TBLK):
            j = min(i + _TBLK, rows)
            dst[i:j] = src_t[:, i:j].T

    if fast:
        def _fill_merged(c):
            _untranspose_into(
                out[c * ROWS_PC : (c + 1) * ROWS_PC], results[c]["outT"]
            )

        jobs = [_POOL.submit(_fill_merged, c) for c in range(N_CORES)]
        step = (N - M) // N_CORES
        for c in range(N_CORES):
            lo = M + c * step
            hi = N if c == N_CORES - 1 else M + (c + 1) * step
            jobs.append(_POOL.submit(_copy_rows, out, h_full, lo, hi))
        for j in jobs:
            j.result()
    else:
        merged = np.empty((M, DIM), np.float32)

        def _fill_merged(c):
            _untranspose_into(
                merged[c * ROWS_PC : (c + 1) * ROWS_PC], results[c]["outT"]
            )

        list(_POOL.map(_fill_merged, range(N_CORES)))
        np.copyto(out, h_full)
        out[idx] = merged
    return out
